# revision 60
# baseline (speedup 1.0000x reference)
"""Depth-aware 3x3 convolution on 8 Trainium2 NeuronCores (Bass, raw engine blocks).

out[b,o,h,w] = sum_{c,kh,kw} weight[o,c,kh,kw] * x[b,c,h+kh-1,w+kw-1]
                             * exp(-8.3*|depth[b,h,w] - depth[b,h+kh-1,w+kw-1]|)

Sharding: core = 2*b + (h >= 128); each core computes a [32, 128, 256] output
slab from a 130-row padded input frame (1-row halo from the host slice).

v3 pipeline (bf16 on-chip, 2-tile groups of 16 rows = 4096 px):
  A. sim phase, pipelined per tap-row t: |dc-dk| (DVE, fp16) -> exp (ACT,
     bf16) -> DMA to DRAM simd[9, 32768] -> broadcasts for groups 0/1.
  B. x3 (3 column-shift blocks on partitions, bf16) loaded as 2 upfront
     quad-chunks [96, 66*258]; no reloads.
  C. loop over 8 groups g (double-buffered):
     - SP   : 8 broadcast DMAs (center tap is a constant-1 region memset once
              by Pool): simd row -> simrep3 [32, 4096] bf16
     - DVE  : per t one mult [96, 16, 256]: xm3 = x3(rows) * simrep3 (2x mode)
     - PE   : per tile (2 per group), 3x4 accumulating matmuls K=96 N=512
     - ACT  : psum -> out_sb bf16; Pool stores [32, 4096] per group.
"""
import sys

import numpy as np

sys.path.insert(0, "/opt/trn_rl_repo")

import concourse.bass as bass
import concourse.mybir as mybir
from concourse.bass_utils import run_bass_kernel_spmd

try:
    import ml_dtypes

    BF16_NP = ml_dtypes.bfloat16
except ImportError:  # pragma: no cover
    BF16_NP = None

F32 = mybir.dt.float32
F16 = mybir.dt.float16
BF16 = mybir.dt.bfloat16
EXP = mybir.ActivationFunctionType.Exp

B, C, H, W = 4, 32, 256, 256
O = 32
ALPHA = 8.3
R = 128  # output rows per core
WP = W + 2  # padded width
FR = R + 2  # frame rows per core
NPIX = R * W  # 32768
GROWS = 16  # rows per group
GPIX = GROWS * W  # 4096
NG = R // GROWS  # 8 groups
QROWS = 4 * GROWS + 2  # x3 quad-chunk rows (66)
QELEM = QROWS * WP  # 17028
TILE = 2048  # matmul tile (psum buffer)
MMN = 512  # matmul free-dim chunk (ISA max moving elements)


def build_nc():
    nc = bass.Bass("TRN2", target_bir_lowering=False, debug=False, num_devices=8)
    x_in = nc.declare_dram_parameter("x", [32, FR * WP], BF16, isOutput=False)
    dp_in = nc.declare_dram_parameter("dp", [FR, WP], F16, isOutput=False)
    w3_in = nc.declare_dram_parameter("w3", [96, 96], BF16, isOutput=False)
    ones_in = nc.declare_dram_parameter("ones", [1, GPIX], BF16, isOutput=False)
    out_d = nc.declare_dram_parameter("out", [O, NPIX], BF16, isOutput=True)
    simd = nc.dram_tensor("simd", [9, NPIX], BF16)

    from contextlib import ExitStack

    ctx = ExitStack()
    with ctx:
        d_sb = ctx.enter_context(nc.sbuf_tensor([128, 3 * WP], F16))
        adiff9 = ctx.enter_context(nc.sbuf_tensor([128, 9 * W], F16))
        sim9 = ctx.enter_context(nc.sbuf_tensor([128, 9 * W], BF16))
        w3_sb = ctx.enter_context(nc.sbuf_tensor([96, 96], BF16))
        x3c = ctx.enter_context(nc.sbuf_tensor([96, 2 * QELEM], BF16))
        simrep3 = ctx.enter_context(nc.sbuf_tensor([96, 2 * 3 * GPIX], BF16))
        xm3 = ctx.enter_context(nc.sbuf_tensor([96, 2 * 3 * GPIX], BF16))
        out_sb = ctx.enter_context(nc.sbuf_tensor([32, 2 * GPIX], BF16))
        psum = ctx.enter_context(nc.psum_tensor([32, 2 * TILE], F32))
        ld_sem = ctx.enter_context(nc.semaphore("ld_sem"))
        w_sem = ctx.enter_context(nc.semaphore("w_sem"))
        # per-chunk load / shift-copy sems (loads are unpaced, any order)
        x8 = [ctx.enter_context(nc.semaphore(f"x8_{i}")) for i in range(NG)]
        xc8 = [ctx.enter_context(nc.semaphore(f"xc8_{i}")) for i in range(NG)]
        # per-t broadcast sems, per buffer parity
        bt = [
            [ctx.enter_context(nc.semaphore(f"bt{t}{p}")) for p in range(2)]
            for t in range(3)
        ]
        sim_dve = ctx.enter_context(nc.semaphore("sim_dve"))
        act_exp = ctx.enter_context(nc.semaphore("act_exp"))
        # per-t sim store sems
        sst = [ctx.enter_context(nc.semaphore(f"sst{t}")) for t in range(3)]
        ones_sem = ctx.enter_context(nc.semaphore("ones_sem"))
        mod_sem = ctx.enter_context(nc.semaphore("mod_sem"))
        pe_sem = ctx.enter_context(nc.semaphore("pe_sem"))
        cp_sem = ctx.enter_context(nc.semaphore("cp_sem"))
        st_e = ctx.enter_context(nc.semaphore("st_e"))
        st_o = ctx.enter_context(nc.semaphore("st_o"))
        block = ctx.enter_context(nc.Block())

        # x3c row view: [p, quad, row, col]
        x3c_r = x3c.ap().rearrange("p (q r w) -> p q r w", q=2, w=WP)
        # xm3 / simrep3 group-tap views: [p, buf, t, px]
        xm3_v = xm3.ap().rearrange("p (b t px) -> p b t px", b=2, px=GPIX)
        sr3_v = simrep3.ap().rearrange("p (b t px) -> p b t px", b=2, px=GPIX)
        # number of broadcast DMAs per (g, t): center tap skipped
        NBC = [3, 2, 3]

        @block.sync
        def _(sync: bass.BassEngine):
            # startup loads: d (3 row-shifted views), w3, both x3 quads
            for t in range(3):
                sync.dma_start(
                    d_sb[:, t * WP : (t + 1) * WP], dp_in[t : t + 128, :]
                ).then_inc(ld_sem, 16)
            sync.dma_start(w3_sb[:], w3_in[:]).then_inc(w_sem, 16)

            # x group chunks into the j=1 partition block of the quad layout.
            # Disjoint rows: 16 per chunk, 18 for the last chunk of a quad
            # (group g's 18-row window spans chunks g and g+1).
            def x3load(g):
                q, m = g // 4, g % 4
                n = (18 if m == 3 else 16) * WP
                dst0 = q * QELEM + 16 * m * WP
                src0 = (16 * g) * WP
                sync.dma_start(
                    x3c[32:64, dst0 : dst0 + n],
                    x_in[:, src0 : src0 + n],
                ).then_inc(x8[g], 16)

            def simstore(t):
                sync.wait_ge(act_exp, t + 1)
                sync.dma_start(
                    simd.ap()[3 * t : 3 * t + 3].rearrange("k (r w) -> r k w", w=W),
                    sim9.ap()[:, 3 * t * W : (3 * t + 3) * W].rearrange(
                        "p (k w) -> p k w", w=W
                    ),
                ).then_inc(sst[t], 16)

            # x loads fill SP's wait gaps; sim stores pipelined per tap-row
            x3load(0)
            x3load(1)
            x3load(2)
            simstore(0)
            x3load(3)
            x3load(4)

            def bcast(g, t):
                # broadcasts (t, j): simd row 3t+j -> simrep3[32j:32j+32, t].
                # (t=1, j=1) is the center tap: sim == 1, region pre-filled
                # by a Pool memset and never overwritten.
                for j in range(3):
                    if t == 1 and j == 1:
                        continue
                    sync.dma_start(
                        simrep3[
                            32 * j : 32 * (j + 1),
                            (g % 2) * 3 * GPIX
                            + t * GPIX : (g % 2) * 3 * GPIX
                            + (t + 1) * GPIX,
                        ],
                        simd[
                            3 * t + j : 3 * t + j + 1,
                            g * GPIX : (g + 1) * GPIX,
                        ].to_broadcast((32, GPIX)),
                    ).then_inc(bt[t][g % 2], 16)

            # head broadcasts for groups 0/1, per t as sim rows land,
            # remaining x loads interleaved into the wait gaps
            sync.wait_ge(sst[0], 16)
            bcast(0, 0)
            bcast(1, 0)
            # center-tap (t=1, j=1) simrep3 regions: constant 1.0, broadcast
            # once per buffer, never overwritten
            for b in range(2):
                sync.dma_start(
                    simrep3[32:64, b * 3 * GPIX + GPIX : b * 3 * GPIX + 2 * GPIX],
                    ones_in[0:1, :].to_broadcast((32, GPIX)),
                ).then_inc(ones_sem, 16)
            x3load(5)
            simstore(1)
            x3load(6)
            sync.wait_ge(sst[1], 16)
            bcast(0, 1)
            bcast(1, 1)
            x3load(7)
            simstore(2)
            sync.wait_ge(sst[2], 16)
            bcast(0, 2)
            bcast(1, 2)
            # steady state: broadcast for g+2 (paced by DVE), store g
            for g in range(NG):
                if g + 2 < NG:
                    sync.wait_ge(mod_sem, 3 * g + 3)
                    for t in range(3):
                        bcast(g + 2, t)
                sync.wait_ge(cp_sem, 2 * g + 2)
                sync.dma_start(
                    out_d[:, g * GPIX : (g + 1) * GPIX],
                    out_sb[:, (g % 2) * GPIX : (g % 2 + 1) * GPIX],
                ).then_inc(st_e if g % 2 == 0 else st_o, 16)

        @block.gpsimd
        def _(pool):
            # column-shift copies (SBUF->SBUF, no HBM traffic): j=0/j=2
            # partition blocks are the j=1 block shifted by -/+1 element,
            # chunk-local (first element of j=0 / last of j=2 per chunk is
            # left unwritten: column 0/257, never read by the modulation).
            def xshift(g):
                q, m = g // 4, g % 4
                n = (18 if m == 3 else 16) * WP - 1
                dst0 = q * QELEM + 16 * m * WP
                pool.wait_ge(x8[g], 16)
                pool.dma_start(
                    x3c[0:32, dst0 + 1 : dst0 + 1 + n],
                    x3c[32:64, dst0 : dst0 + n],
                ).then_inc(xc8[g], 16)
                pool.dma_start(
                    x3c[64:96, dst0 : dst0 + n],
                    x3c[32:64, dst0 + 1 : dst0 + 1 + n],
                ).then_inc(xc8[g], 16)

            for g in range(NG):
                xshift(g)

        @block.vector
        def _(vector):
            # sim phase: diff + abs, one drain, per-t completion increments
            vector.wait_ge(ld_sem, 48)
            for k in range(9):
                vector.tensor_sub(
                    adiff9[:, k * W : (k + 1) * W],
                    d_sb[:, WP + 1 : WP + 1 + W],
                    d_sb[:, (k // 3) * WP + k % 3 : (k // 3) * WP + k % 3 + W],
                )
            vector.drain()
            for k in range(9):
                vector.scalar_tensor_tensor(
                    adiff9[:, k * W : (k + 1) * W],
                    adiff9[:, k * W : (k + 1) * W],
                    -1.0,
                    adiff9[:, k * W : (k + 1) * W],
                    op0=mybir.AluOpType.mult,
                    op1=mybir.AluOpType.max,
                ).then_inc(sim_dve, 1)
            # modulation loop: per (g, t) one [96, 16, 256] mult
            for g in range(NG):
                bi = g % 2
                q, rb = g // 4, 16 * (g % 4)
                for t in range(3):
                    if t == 0:
                        # group g's 18-row window spans chunks g and g+1
                        # (chunk g alone when it is the last of its quad)
                        chunks = (g,) if g % 4 == 3 else (g, g + 1)
                        for l in chunks:
                            vector.wait_ge(xc8[l], 32)
                    # all bcasts of this (g, t); completion order across
                    # queues is not guaranteed, so wait on the per-t sum
                    vector.wait_ge(bt[t][bi], 16 * NBC[t] * (g // 2 + 1))
                    if g == 0 and t == 1:
                        vector.wait_ge(ones_sem, 32)
                    if g >= 2:
                        vector.wait_ge(pe_sem, 6 * g - 8 + t)
                    vector.tensor_mul(
                        xm3_v[:, bi, t].rearrange("p (r w) -> p r w", w=W),
                        x3c_r[:, q, rb + t : rb + t + GROWS, 1 : 1 + W],
                        sr3_v[:, bi, t].rearrange("p (r w) -> p r w", w=W),
                    ).then_inc(mod_sem, 1)

        @block.tensor
        def _(tensor):
            tensor.wait_ge(w_sem, 16)
            for i in range(2 * NG):
                g, h = i // 2, i % 2
                for t in range(3):
                    tensor.wait_ge(mod_sem, 3 * g + t + 1)
                    if t == 0 and i >= 2:
                        tensor.wait_ge(cp_sem, i - 1)
                    base = (g % 2) * 3 * GPIX + t * GPIX + h * TILE
                    for qq in range(TILE // MMN):
                        mm = tensor.matmul(
                            psum[
                                :,
                                (i % 2) * TILE
                                + qq * MMN : (i % 2) * TILE
                                + (qq + 1) * MMN,
                            ],
                            w3_sb[:, 32 * t : 32 * (t + 1)],
                            xm3[:, base + qq * MMN : base + (qq + 1) * MMN],
                            start=(t == 0),
                            stop=(t == 2),
                        )
                        if qq == TILE // MMN - 1:
                            mm.then_inc(pe_sem, 1)

        @block.scalar
        def _(scalar):
            # exp per tap-row t (bf16 out)
            for t in range(3):
                scalar.wait_ge(sim_dve, 3 * (t + 1))
                scalar.activation(
                    sim9[:, 3 * t * W : (3 * t + 3) * W],
                    adiff9[:, 3 * t * W : (3 * t + 3) * W],
                    EXP,
                    scale=-ALPHA,
                ).then_inc(act_exp, 1)
            # psum -> sbuf copies (bf16)
            for i in range(2 * NG):
                g, h = i // 2, i % 2
                scalar.wait_ge(pe_sem, 3 * i + 3)
                if g >= 2:
                    scalar.wait_ge(st_e if g % 2 == 0 else st_o, 16 * (g // 2))
                scalar.copy(
                    out_sb[
                        :,
                        (g % 2) * GPIX + h * TILE : (g % 2) * GPIX + h * TILE + TILE,
                    ],
                    psum[:, (i % 2) * TILE : (i % 2 + 1) * TILE],
                ).then_inc(cp_sem, 1)

    return nc


_NC_CACHE = None


def _get_nc():
    global _NC_CACHE
    if _NC_CACHE is None:
        _NC_CACHE = build_nc()
    return _NC_CACHE


def _prep_core(x, depth, core):
    b, half = core // 2, core % 2
    r0 = half * R
    # padded frame [C, FR, WP]: image rows r0-1 .. r0+R, zero-padded
    xpad = np.zeros((C, FR, WP), dtype=np.float32)
    dpad = np.zeros((FR, WP), dtype=np.float32)
    lo, hi = r0 - 1, r0 + R + 1
    slo, shi = max(lo, 0), min(hi, H)
    xpad[:, slo - lo : shi - lo, 1 : 1 + W] = x[b, :, slo:shi, :]
    dpad[slo - lo : shi - lo, 1 : 1 + W] = depth[b, 0, slo:shi, :]
    return {
        "x": xpad.reshape(C, FR * WP).astype(BF16_NP),
        "dp": dpad.astype(np.float16),
    }


def make_in_maps(x, depth, weight):
    x = np.ascontiguousarray(x, dtype=np.float32)
    depth = np.ascontiguousarray(depth, dtype=np.float32)
    weight = np.ascontiguousarray(weight, dtype=np.float32)
    # w3[32j + c, 32t + o] = weight[o, c, t, j]
    w3 = (
        np.transpose(weight, (3, 1, 2, 0)).reshape(96, 96).astype(BF16_NP)
    )
    ones = np.ones((1, GPIX), dtype=BF16_NP)
    in_maps = []
    for core in range(8):
        m = _prep_core(x, depth, core)
        m["w3"] = w3
        m["ones"] = ones
        in_maps.append(m)
    return in_maps


def kernel(x, depth, weight):
    in_maps = make_in_maps(x, depth, weight)
    nc = _get_nc()
    res = run_bass_kernel_spmd(nc, in_maps, list(range(8)))

    out = np.empty((B, O, H, W), dtype=np.float32)
    for core in range(8):
        b, half = core // 2, core % 2
        out[b, :, half * R : (half + 1) * R, :] = (
            np.asarray(res.results[core]["out"]).astype(np.float32).reshape(O, R, W)
        )
    return out


# revision 65
# speedup vs baseline: 1.0268x; 1.0268x over previous
"""Depth-aware 3x3 convolution on 8 Trainium2 NeuronCores (Bass, raw engine blocks).

out[b,o,h,w] = sum_{c,kh,kw} weight[o,c,kh,kw] * x[b,c,h+kh-1,w+kw-1]
                             * exp(-8.3*|depth[b,h,w] - depth[b,h+kh-1,w+kw-1]|)

Sharding: core = 2*b + (h >= 128); each core computes a [32, 128, 256] output
slab from a 130-row padded input frame (1-row halo from the host slice).

v3 pipeline (bf16 on-chip, 2-tile groups of 16 rows = 4096 px):
  A. sim phase, pipelined per tap-row t: |dc-dk| (DVE, fp16) -> exp (ACT,
     bf16) -> DMA to DRAM simd[9, 32768] -> broadcasts for groups 0/1.
  B. x3 (3 column-shift blocks on partitions, bf16) loaded as 2 upfront
     quad-chunks [96, 66*258]; no reloads.
  C. loop over 8 groups g (double-buffered):
     - SP   : 8 broadcast DMAs (center tap is a constant-1 region memset once
              by Pool): simd row -> simrep3 [32, 4096] bf16
     - DVE  : per t one mult [96, 16, 256]: xm3 = x3(rows) * simrep3 (2x mode)
     - PE   : per tile (2 per group), 3x4 accumulating matmuls K=96 N=512
     - ACT  : psum -> out_sb bf16; Pool stores [32, 4096] per group.
"""
import sys

import numpy as np

sys.path.insert(0, "/opt/trn_rl_repo")

import concourse.bass as bass
import concourse.mybir as mybir
from concourse.bass_utils import run_bass_kernel_spmd

try:
    import ml_dtypes

    BF16_NP = ml_dtypes.bfloat16
except ImportError:  # pragma: no cover
    BF16_NP = None

F32 = mybir.dt.float32
F16 = mybir.dt.float16
BF16 = mybir.dt.bfloat16
EXP = mybir.ActivationFunctionType.Exp

B, C, H, W = 4, 32, 256, 256
O = 32
ALPHA = 8.3
R = 128  # output rows per core
WP = W + 2  # padded width
FR = R + 2  # frame rows per core
NPIX = R * W  # 32768
GROWS = 16  # rows per group
GPIX = GROWS * W  # 4096
NG = R // GROWS  # 8 groups
QROWS = 4 * GROWS + 2  # x3 quad-chunk rows (66)
QELEM = QROWS * WP  # 17028
TILE = 2048  # matmul tile (psum buffer)
MMN = 512  # matmul free-dim chunk (ISA max moving elements)


def build_nc():
    nc = bass.Bass("TRN2", target_bir_lowering=False, debug=False, num_devices=8)
    x_in = nc.declare_dram_parameter("x", [32, FR * WP], BF16, isOutput=False)
    dp_in = nc.declare_dram_parameter("dp", [FR, WP], F16, isOutput=False)
    w3_in = nc.declare_dram_parameter("w3", [96, 96], BF16, isOutput=False)
    out_d = nc.declare_dram_parameter("out", [O, NPIX], BF16, isOutput=True)
    simd = nc.dram_tensor("simd", [9, NPIX], BF16)

    from contextlib import ExitStack

    ctx = ExitStack()
    with ctx:
        d_sb = ctx.enter_context(nc.sbuf_tensor([128, 3 * WP], F16))
        adiff9 = ctx.enter_context(nc.sbuf_tensor([128, 9 * W], F16))
        sim9 = ctx.enter_context(nc.sbuf_tensor([128, 9 * W], BF16))
        w3_sb = ctx.enter_context(nc.sbuf_tensor([96, 96], BF16))
        x3c = ctx.enter_context(nc.sbuf_tensor([96, 2 * QELEM], BF16))
        simrep3 = ctx.enter_context(nc.sbuf_tensor([96, 2 * 3 * GPIX], BF16))
        xm3 = ctx.enter_context(nc.sbuf_tensor([96, 2 * 3 * GPIX], BF16))
        out_sb = ctx.enter_context(nc.sbuf_tensor([32, 2 * GPIX], BF16))
        psum = ctx.enter_context(nc.psum_tensor([32, 2 * TILE], F32))
        ld_sem = ctx.enter_context(nc.semaphore("ld_sem"))
        w_sem = ctx.enter_context(nc.semaphore("w_sem"))
        # per-chunk load / shift-copy sems (loads are unpaced, any order)
        x8 = [ctx.enter_context(nc.semaphore(f"x8_{i}")) for i in range(NG)]
        xc8 = [ctx.enter_context(nc.semaphore(f"xc8_{i}")) for i in range(NG)]
        # per-t broadcast sems, per buffer parity
        bt = [
            [ctx.enter_context(nc.semaphore(f"bt{t}{p}")) for p in range(2)]
            for t in range(3)
        ]
        sim_dve = ctx.enter_context(nc.semaphore("sim_dve"))
        act_exp = ctx.enter_context(nc.semaphore("act_exp"))
        # per-t sim store sems
        sst = [ctx.enter_context(nc.semaphore(f"sst{t}")) for t in range(3)]
        ones_sem = ctx.enter_context(nc.semaphore("ones_sem"))
        mod_sem = ctx.enter_context(nc.semaphore("mod_sem"))
        pe_sem = ctx.enter_context(nc.semaphore("pe_sem"))
        cp_sem = ctx.enter_context(nc.semaphore("cp_sem"))
        st_e = ctx.enter_context(nc.semaphore("st_e"))
        st_o = ctx.enter_context(nc.semaphore("st_o"))
        block = ctx.enter_context(nc.Block())

        # x3c row view: [p, quad, row, col]
        x3c_r = x3c.ap().rearrange("p (q r w) -> p q r w", q=2, w=WP)
        # xm3 / simrep3 group-tap views: [p, buf, t, px]
        xm3_v = xm3.ap().rearrange("p (b t px) -> p b t px", b=2, px=GPIX)
        sr3_v = simrep3.ap().rearrange("p (b t px) -> p b t px", b=2, px=GPIX)
        # number of broadcast DMAs per (g, t): center tap skipped
        NBC = [3, 2, 3]

        @block.sync
        def _(sync: bass.BassEngine):
            # startup loads: d (3 row-shifted views), w3, both x3 quads
            for t in range(3):
                sync.dma_start(
                    d_sb[:, t * WP : (t + 1) * WP], dp_in[t : t + 128, :]
                ).then_inc(ld_sem, 16)
            sync.dma_start(w3_sb[:], w3_in[:]).then_inc(w_sem, 16)

            # x group chunks into the j=1 partition block of the quad layout.
            # Disjoint rows: 16 per chunk, 18 for the last chunk of a quad
            # (group g's 18-row window spans chunks g and g+1).
            def x3load(g):
                q, m = g // 4, g % 4
                n = (18 if m == 3 else 16) * WP
                dst0 = q * QELEM + 16 * m * WP
                src0 = (16 * g) * WP
                sync.dma_start(
                    x3c[32:64, dst0 : dst0 + n],
                    x_in[:, src0 : src0 + n],
                ).then_inc(x8[g], 16)

            def simstore(t):
                sync.wait_ge(act_exp, t + 1)
                sync.dma_start(
                    simd.ap()[3 * t : 3 * t + 3].rearrange("k (r w) -> r k w", w=W),
                    sim9.ap()[:, 3 * t * W : (3 * t + 3) * W].rearrange(
                        "p (k w) -> p k w", w=W
                    ),
                ).then_inc(sst[t], 16)

            # x loads fill SP's wait gaps; sim stores pipelined per tap-row
            x3load(0)
            x3load(1)
            x3load(2)
            simstore(0)
            x3load(3)
            x3load(4)

            def bcast(g, t):
                # broadcasts (t, j): simd row 3t+j -> simrep3[32j:32j+32, t].
                # (t=1, j=1) is the center tap: sim == 1, region pre-filled
                # by a Pool memset and never overwritten.
                for j in range(3):
                    if t == 1 and j == 1:
                        continue
                    sync.dma_start(
                        simrep3[
                            32 * j : 32 * (j + 1),
                            (g % 2) * 3 * GPIX
                            + t * GPIX : (g % 2) * 3 * GPIX
                            + (t + 1) * GPIX,
                        ],
                        simd[
                            3 * t + j : 3 * t + j + 1,
                            g * GPIX : (g + 1) * GPIX,
                        ].to_broadcast((32, GPIX)),
                    ).then_inc(bt[t][g % 2], 16)

            # head broadcasts for groups 0/1, per t as sim rows land,
            # remaining x loads interleaved into the wait gaps
            sync.wait_ge(sst[0], 16)
            bcast(0, 0)
            bcast(1, 0)
            x3load(5)
            simstore(1)
            x3load(6)
            sync.wait_ge(sst[1], 16)
            bcast(0, 1)
            bcast(1, 1)
            x3load(7)
            simstore(2)
            sync.wait_ge(sst[2], 16)
            bcast(0, 2)
            bcast(1, 2)
            # steady state: broadcast for g+2 (paced by DVE), store g
            for g in range(NG):
                if g + 2 < NG:
                    sync.wait_ge(mod_sem, 3 * g + 3)
                    for t in range(3):
                        bcast(g + 2, t)
                sync.wait_ge(cp_sem, 2 * g + 2)
                sync.dma_start(
                    out_d[:, g * GPIX : (g + 1) * GPIX],
                    out_sb[:, (g % 2) * GPIX : (g % 2 + 1) * GPIX],
                ).then_inc(st_e if g % 2 == 0 else st_o, 16)

        @block.gpsimd
        def _(pool):
            # column-shift copies (SBUF->SBUF, no HBM traffic): j=0/j=2
            # partition blocks are the j=1 block shifted by -/+1 element,
            # chunk-local (first element of j=0 / last of j=2 per chunk is
            # left unwritten: column 0/257, never read by the modulation).
            def xshift(g):
                q, m = g // 4, g % 4
                n = (18 if m == 3 else 16) * WP - 1
                dst0 = q * QELEM + 16 * m * WP
                pool.wait_ge(x8[g], 16)
                pool.dma_start(
                    x3c[0:32, dst0 + 1 : dst0 + 1 + n],
                    x3c[32:64, dst0 : dst0 + n],
                ).then_inc(xc8[g], 16)
                pool.dma_start(
                    x3c[64:96, dst0 : dst0 + n],
                    x3c[32:64, dst0 + 1 : dst0 + 1 + n],
                ).then_inc(xc8[g], 16)

            xshift(0)
            # pre-fill center-tap (t=1, j=1) simrep3 regions with 1.0
            for b in range(2):
                pool.memset(
                    simrep3[32:64, b * 3 * GPIX + GPIX : b * 3 * GPIX + 2 * GPIX],
                    1.0,
                ).then_inc(ones_sem, 1)
            for g in range(1, NG):
                xshift(g)

        @block.vector
        def _(vector):
            # sim phase: diff + abs, one drain, per-t completion increments
            vector.wait_ge(ld_sem, 48)
            for k in range(9):
                vector.tensor_sub(
                    adiff9[:, k * W : (k + 1) * W],
                    d_sb[:, WP + 1 : WP + 1 + W],
                    d_sb[:, (k // 3) * WP + k % 3 : (k // 3) * WP + k % 3 + W],
                )
            vector.drain()
            for k in range(9):
                vector.scalar_tensor_tensor(
                    adiff9[:, k * W : (k + 1) * W],
                    adiff9[:, k * W : (k + 1) * W],
                    -1.0,
                    adiff9[:, k * W : (k + 1) * W],
                    op0=mybir.AluOpType.mult,
                    op1=mybir.AluOpType.max,
                ).then_inc(sim_dve, 1)
            # modulation loop: per (g, t) one [96, 16, 256] mult
            for g in range(NG):
                bi = g % 2
                q, rb = g // 4, 16 * (g % 4)
                for t in range(3):
                    if t == 0:
                        # group g's 18-row window spans chunks g and g+1
                        # (chunk g alone when it is the last of its quad)
                        chunks = (g,) if g % 4 == 3 else (g, g + 1)
                        for l in chunks:
                            vector.wait_ge(xc8[l], 32)
                    # all bcasts of this (g, t); completion order across
                    # queues is not guaranteed, so wait on the per-t sum
                    vector.wait_ge(bt[t][bi], 16 * NBC[t] * (g // 2 + 1))
                    if g == 0 and t == 1:
                        vector.wait_ge(ones_sem, 2)
                    if g >= 2:
                        vector.wait_ge(pe_sem, 6 * g - 8 + t)
                    vector.tensor_mul(
                        xm3_v[:, bi, t].rearrange("p (r w) -> p r w", w=W),
                        x3c_r[:, q, rb + t : rb + t + GROWS, 1 : 1 + W],
                        sr3_v[:, bi, t].rearrange("p (r w) -> p r w", w=W),
                    ).then_inc(mod_sem, 1)

        @block.tensor
        def _(tensor):
            tensor.wait_ge(w_sem, 16)
            for i in range(2 * NG):
                g, h = i // 2, i % 2
                for t in range(3):
                    tensor.wait_ge(mod_sem, 3 * g + t + 1)
                    if t == 0 and i >= 2:
                        tensor.wait_ge(cp_sem, i - 1)
                    base = (g % 2) * 3 * GPIX + t * GPIX + h * TILE
                    for qq in range(TILE // MMN):
                        mm = tensor.matmul(
                            psum[
                                :,
                                (i % 2) * TILE
                                + qq * MMN : (i % 2) * TILE
                                + (qq + 1) * MMN,
                            ],
                            w3_sb[:, 32 * t : 32 * (t + 1)],
                            xm3[:, base + qq * MMN : base + (qq + 1) * MMN],
                            start=(t == 0),
                            stop=(t == 2),
                        )
                        if qq == TILE // MMN - 1:
                            mm.then_inc(pe_sem, 1)

        @block.scalar
        def _(scalar):
            # exp per tap-row t (bf16 out)
            for t in range(3):
                scalar.wait_ge(sim_dve, 3 * (t + 1))
                scalar.activation(
                    sim9[:, 3 * t * W : (3 * t + 3) * W],
                    adiff9[:, 3 * t * W : (3 * t + 3) * W],
                    EXP,
                    scale=-ALPHA,
                ).then_inc(act_exp, 1)
            # psum -> sbuf copies (bf16)
            for i in range(2 * NG):
                g, h = i // 2, i % 2
                scalar.wait_ge(pe_sem, 3 * i + 3)
                if g >= 2:
                    scalar.wait_ge(st_e if g % 2 == 0 else st_o, 16 * (g // 2))
                scalar.copy(
                    out_sb[
                        :,
                        (g % 2) * GPIX + h * TILE : (g % 2) * GPIX + h * TILE + TILE,
                    ],
                    psum[:, (i % 2) * TILE : (i % 2 + 1) * TILE],
                ).then_inc(cp_sem, 1)

    return nc


_NC_CACHE = None


def _get_nc():
    global _NC_CACHE
    if _NC_CACHE is None:
        _NC_CACHE = build_nc()
    return _NC_CACHE


def _prep_core(x, depth, core):
    b, half = core // 2, core % 2
    r0 = half * R
    # padded frame [C, FR, WP]: image rows r0-1 .. r0+R, zero-padded
    xpad = np.zeros((C, FR, WP), dtype=np.float32)
    dpad = np.zeros((FR, WP), dtype=np.float32)
    lo, hi = r0 - 1, r0 + R + 1
    slo, shi = max(lo, 0), min(hi, H)
    xpad[:, slo - lo : shi - lo, 1 : 1 + W] = x[b, :, slo:shi, :]
    dpad[slo - lo : shi - lo, 1 : 1 + W] = depth[b, 0, slo:shi, :]
    return {
        "x": xpad.reshape(C, FR * WP).astype(BF16_NP),
        "dp": dpad.astype(np.float16),
    }


def make_in_maps(x, depth, weight):
    x = np.ascontiguousarray(x, dtype=np.float32)
    depth = np.ascontiguousarray(depth, dtype=np.float32)
    weight = np.ascontiguousarray(weight, dtype=np.float32)
    # w3[32j + c, 32t + o] = weight[o, c, t, j]
    w3 = (
        np.transpose(weight, (3, 1, 2, 0)).reshape(96, 96).astype(BF16_NP)
    )
    in_maps = []
    for core in range(8):
        m = _prep_core(x, depth, core)
        m["w3"] = w3
        in_maps.append(m)
    return in_maps


def kernel(x, depth, weight):
    in_maps = make_in_maps(x, depth, weight)
    nc = _get_nc()
    res = run_bass_kernel_spmd(nc, in_maps, list(range(8)))

    out = np.empty((B, O, H, W), dtype=np.float32)
    for core in range(8):
        b, half = core // 2, core % 2
        out[b, :, half * R : (half + 1) * R, :] = (
            np.asarray(res.results[core]["out"]).astype(np.float32).reshape(O, R, W)
        )
    return out
